# revision 7
# baseline (speedup 1.0000x reference)
"""Trainium2 Bass kernel for nn_Attention_Layer (B=4, S=2048, D=1024, fp32).

Sharding: 8 cores = 4 batches x 2 query-halves. Each core computes K/V for
its whole batch (from x^T, pre-transposed on host) and attention for its
1024-query half. Scores are built transposed ([k, q] layout) so the softmax
denominator folds into a per-partition scalar at the output, and the
attn @ V contraction needs no on-device transpose of the attention matrix.

Compute dtypes: projections and scores run the PE in float32r (full-rate
fp32 path); exp runs on ACT in fp32; the attention-weights @ V product runs
in bf16 (weights are probabilities, V rounding averages out).

Loop structure keeps one stationary weight-load per TWO 512-wide moving
matmuls (fp32r LDWEIGHTS is 1.5 cyc/col — letting it serve two matmuls
keeps the PE queue fed).
"""

import numpy as np

import concourse.bass as bass
import concourse.mybir as mybir
import concourse.tile as tile
from concourse import bacc
from concourse.bass_utils import run_bass_kernel_spmd

B, S, D = 4, 2048, 1024
P = 128
HALF = S // 2            # queries per core; also the k-half processed per phase
EO = D // P              # 8 e-tiles (feature dim outer)
DO = D // P              # 8 d-tiles (contraction outer)
KO = S // P              # 16 k-tiles (global)
QT = HALF // P           # 8 q-tiles per core
SCALE = 1.0 / np.sqrt(D)

F32 = mybir.dt.float32
F32R = mybir.dt.float32r
BF16 = mybir.dt.bfloat16


def build_nc():
    nc = bacc.Bacc("TRN2", target_bir_lowering=False)

    xT = nc.dram_tensor("xT", [D, S], F32R, kind="ExternalInput")
    Wk = nc.dram_tensor("Wk", [D, D], F32R, kind="ExternalInput")
    Wq = nc.dram_tensor("Wq", [D, D], F32R, kind="ExternalInput")
    Wv = nc.dram_tensor("Wv", [D, D], F32R, kind="ExternalInput")
    bkT = nc.dram_tensor("bkT", [P, EO], F32, kind="ExternalInput")
    bqT = nc.dram_tensor("bqT", [P, EO], F32, kind="ExternalInput")
    bv = nc.dram_tensor("bv", [P, D], BF16, kind="ExternalInput")
    y = nc.dram_tensor("y", [HALF, D], F32, kind="ExternalOutput")

    xTr = xT.ap().rearrange("(do p) s -> p do s", p=P)
    Wkr = Wk.ap().rearrange("(do p) e -> p do e", p=P)
    Wqr = Wq.ap().rearrange("(do p) e -> p do e", p=P)
    Wvr = Wv.ap().rearrange("(do p) e -> p do e", p=P)

    with tile.TileContext(nc) as tc:
        with (
            tc.tile_pool(name="xts", bufs=2) as xts_pool,       # 32KB
            tc.tile_pool(name="wke", bufs=2) as wke_pool,       # 8KB
            tc.tile_pool(name="wve", bufs=1) as wve_pool,       # 32KB
            tc.tile_pool(name="kt", bufs=1) as k_pool,          # 32KB
            tc.tile_pool(name="qt", bufs=1) as q_pool,          # 32KB
            tc.tile_pool(name="vt", bufs=1) as v_pool,          # 32KB
            tc.tile_pool(name="pt", bufs=1) as p_pool,          # 32KB
            tc.tile_pool(name="outp", bufs=2) as out_pool,      # 4KB
            tc.tile_pool(name="small", bufs=1) as small_pool,
            tc.tile_pool(name="ps", bufs=6, space="PSUM") as ps_pool,
            tc.tile_pool(name="avz", bufs=1, space="PSUM") as avz_pool,
        ):
            bk_sb = small_pool.tile([P, EO], F32, tag="bk")
            bq_sb = small_pool.tile([P, EO], F32, tag="bq")
            bv_sb = small_pool.tile([P, D], BF16, tag="bv")
            ones_sb = small_pool.tile([P, 1], BF16, tag="ones")
            rz_sb = small_pool.tile([P, QT], F32, tag="rz")
            nc.vector.memset(ones_sb[:], 1.0)

            def emit_deferred_small_loads():
                nc.sync.dma_start(bk_sb[:], bkT[:, :])
                nc.sync.dma_start(bq_sb[:], bqT[:, :])
                nc.sync.dma_start(bv_sb[:], bv[:, :])

            # keep the PE busy (HAM warm) while the first x/W DMAs land
            warm_ps = avz_pool.tile([1, 8], F32, tag="warm")
            for _ in range(120):
                nc.tensor.matmul(
                    warm_ps[:, 0:1], ones_sb[:], ones_sb[:],
                    start=True, stop=True,
                )

            q_sb = q_pool.tile([P, EO, HALF], F32R, tag="qt")
            v_sb = v_pool.tile([P, KO, D], BF16, tag="vt")
            p_sb = p_pool.tile([P, KO, D], BF16, tag="pt")

            # ---- projections + scores, one k-half at a time --------------
            # query half is always s in [0, HALF) after the host swap.
            for kh in range(2):
                k_sb = k_pool.tile([P, EO, HALF], F32R, tag="kt")
                xq = []
                for sq in range(2):
                    t = xts_pool.tile([P, DO, 512], F32R, tag="xts")
                    for do in range(DO):   # split DMA across queues
                        nc.sync.dma_start(
                            t[:, do, :],
                            xTr[:, do, kh * HALF + sq * 512 : kh * HALF + sq * 512 + 512],
                        )
                    xq.append(t)
                if kh == 0:
                    emit_deferred_small_loads()
                # K^T (and Q^T in the query half), one 512-wide s-block pass
                # at a time so the first matmul group only needs quarter 0.
                projs = [(Wkr, bk_sb, k_sb)]
                if kh == 0:
                    projs.append((Wqr, bq_sb, q_sb))
                for sq in range(2):
                    for Wr, b_sb, dst_sb in projs:
                        for eo in range(EO):
                            wke = wke_pool.tile([P, DO, P], F32R, tag="wke")
                            nc.sync.dma_start(
                                wke[:], Wr[:, :, eo * P : (eo + 1) * P]
                            )
                            ps0 = ps_pool.tile([P, 512], F32, tag="ps")
                            for do in range(DO):
                                nc.tensor.matmul(
                                    ps0[:], wke[:, do, :], xq[sq][:, do, :],
                                    start=(do == 0), stop=(do == DO - 1),
                                )
                            nc.vector.tensor_scalar_add(
                                dst_sb[:, eo, sq * 512 : sq * 512 + 512],
                                ps0[:],
                                b_sb[:, eo : eo + 1],
                            )
                # V[s-half, :]: one ldweights per (ktl, do) serving both
                # 512-wide e-blocks; Wv d-slices resident for the half.
                wve = wve_pool.tile([P, DO, D], F32R, tag="wve")
                for do in range(DO):
                    nc.sync.dma_start(wve[:, do, :], Wvr[:, do, :])
                for ktl in range(8):
                    ko = kh * 8 + ktl
                    ps0 = ps_pool.tile([P, 512], F32, tag="ps")
                    ps1 = ps_pool.tile([P, 512], F32, tag="ps")
                    for do in range(DO):
                        xkt = xq[ktl // 4][:, do, (ktl % 4) * P : (ktl % 4) * P + P]
                        nc.tensor.matmul(
                            ps0[:], xkt, wve[:, do, 0:512],
                            start=(do == 0), stop=(do == DO - 1),
                        )
                        nc.tensor.matmul(
                            ps1[:], xkt, wve[:, do, 512:1024],
                            start=(do == 0), stop=(do == DO - 1),
                        )
                    nc.vector.tensor_tensor(
                        v_sb[:, ko, 0:512], ps0[:], bv_sb[:, 0:512],
                        mybir.AluOpType.add,
                    )
                    nc.vector.tensor_tensor(
                        v_sb[:, ko, 512:1024], ps1[:], bv_sb[:, 512:1024],
                        mybir.AluOpType.add,
                    )
                # scores^T for this k-half: exp(K^T.T @ Q^T / sqrt(D));
                # one ldweights per (ktl, eo) serving both q-blocks.
                for ktl in range(8):
                    ko = kh * 8 + ktl
                    ps0 = ps_pool.tile([P, 512], F32, tag="ps")
                    ps1 = ps_pool.tile([P, 512], F32, tag="ps")
                    for eo in range(EO):
                        kt_ap = k_sb[:, eo, ktl * P : (ktl + 1) * P]
                        nc.tensor.matmul(
                            ps0[:], kt_ap, q_sb[:, eo, 0:512],
                            start=(eo == 0), stop=(eo == EO - 1),
                        )
                        nc.tensor.matmul(
                            ps1[:], kt_ap, q_sb[:, eo, 512:1024],
                            start=(eo == 0), stop=(eo == EO - 1),
                        )
                    nc.scalar.activation(
                        p_sb[:, ko, 0:512], ps0[:],
                        mybir.ActivationFunctionType.Exp, scale=float(SCALE),
                    )
                    nc.scalar.activation(
                        p_sb[:, ko, 512:1024], ps1[:],
                        mybir.ActivationFunctionType.Exp, scale=float(SCALE),
                    )

            # ---- attention output: (P^T.T @ V) * (1/Z) -------------------
            zt = avz_pool.tile([P, QT], F32, tag="avz")
            for qt in range(QT):
                av0 = ps_pool.tile([P, 512], F32, tag="ps")
                av1 = ps_pool.tile([P, 512], F32, tag="ps")
                for ko in range(KO):
                    lhs = p_sb[:, ko, qt * P : (qt + 1) * P]
                    nc.tensor.matmul(
                        av0[:], lhs, v_sb[:, ko, 0:512],
                        start=(ko == 0), stop=(ko == KO - 1),
                    )
                    nc.tensor.matmul(
                        av1[:], lhs, v_sb[:, ko, 512:1024],
                        start=(ko == 0), stop=(ko == KO - 1),
                    )
                    nc.tensor.matmul(
                        zt[:, qt : qt + 1], lhs, ones_sb[:],
                        start=(ko == 0), stop=(ko == KO - 1),
                    )
                nc.vector.reciprocal(rz_sb[:, qt : qt + 1], zt[:, qt : qt + 1])
                o0 = out_pool.tile([P, 512], F32, tag="outp")
                o1 = out_pool.tile([P, 512], F32, tag="outp")
                nc.vector.tensor_scalar_mul(o0[:], av0[:], rz_sb[:, qt : qt + 1])
                nc.vector.tensor_scalar_mul(o1[:], av1[:], rz_sb[:, qt : qt + 1])
                nc.sync.dma_start(y[qt * P : (qt + 1) * P, 0:512], o0[:])
                nc.sync.dma_start(y[qt * P : (qt + 1) * P, 512:1024], o1[:])

    nc.finalize()
    return nc


_NC_CACHE = None


def make_in_maps(x, Wk, bk, Wq, bq, Wv, bv):
    import ml_dtypes

    x = np.asarray(x, dtype=np.float32)
    Wk = np.ascontiguousarray(np.asarray(Wk, np.float32))
    Wq = np.ascontiguousarray(np.asarray(Wq, np.float32))
    Wv = np.ascontiguousarray(np.asarray(Wv, np.float32))
    bkT = np.ascontiguousarray(np.asarray(bk, np.float32).reshape(EO, P).T)
    bqT = np.ascontiguousarray(np.asarray(bq, np.float32).reshape(EO, P).T)
    bv2 = np.ascontiguousarray(
        np.broadcast_to(
            np.asarray(bv, np.float32).reshape(1, D), (P, D)
        ).astype(ml_dtypes.bfloat16)
    )

    in_maps = []
    for c in range(8):
        b, h = c // 2, c % 2
        xTb = np.ascontiguousarray(x[b].T)          # [D, S]
        if h == 1:
            # swap the s-halves so this core's query half is always first
            xTb = np.ascontiguousarray(
                np.concatenate([xTb[:, HALF:], xTb[:, :HALF]], axis=1)
            )
        in_maps.append(
            {
                "xT": xTb,
                "Wk": Wk, "Wq": Wq, "Wv": Wv,
                "bkT": bkT, "bqT": bqT, "bv": bv2,
            }
        )
    return in_maps


def gather_out(results):
    out = np.empty((B, S, D), dtype=np.float32)
    for c in range(8):
        b, h = c // 2, c % 2
        out[b, h * HALF : (h + 1) * HALF, :] = results[c]["y"]
    return out


def kernel(x, Wk, bk, Wq, bq, Wv, bv):
    global _NC_CACHE
    if _NC_CACHE is None:
        _NC_CACHE = build_nc()
    in_maps = make_in_maps(x, Wk, bk, Wq, bq, Wv, bv)
    res = run_bass_kernel_spmd(_NC_CACHE, in_maps, list(range(8)))
    return gather_out(res.results)


# revision 8
# speedup vs baseline: 1.2537x; 1.2537x over previous
"""Trainium2 Bass kernel for nn_Attention_Layer (B=4, S=2048, D=1024, fp32).

Sharding: 8 cores = 4 batches x 2 query-halves. Each core computes K/V for
its whole batch (from x^T, pre-transposed on host) and attention for its
1024-query half. Scores are built transposed ([k, q] layout) so the softmax
denominator folds into a per-partition scalar at the output, and the
attn @ V contraction needs no on-device transpose of the attention matrix.

Compute dtypes: projections and scores run the PE in float32r (full-rate
fp32 path); exp runs on ACT in fp32; the attention-weights @ V product runs
in bf16 (weights are probabilities, V rounding averages out).

Loop structure keeps one stationary weight-load per TWO 512-wide moving
matmuls (fp32r LDWEIGHTS is 1.5 cyc/col — letting it serve two matmuls
keeps the PE queue fed).
"""

import numpy as np

import concourse.bass as bass
import concourse.mybir as mybir
import concourse.tile as tile
from concourse import bacc
from concourse.bass_utils import run_bass_kernel_spmd

B, S, D = 4, 2048, 1024
P = 128
HALF = S // 2            # queries per core; also the k-half processed per phase
EO = D // P              # 8 e-tiles (feature dim outer)
DO = D // P              # 8 d-tiles (contraction outer)
KO = S // P              # 16 k-tiles (global)
QT = HALF // P           # 8 q-tiles per core
SCALE = 1.0 / np.sqrt(D)

F32 = mybir.dt.float32
F32R = mybir.dt.float32r
BF16 = mybir.dt.bfloat16


def build_nc():
    nc = bacc.Bacc("TRN2", target_bir_lowering=False)

    xT = nc.dram_tensor("xT", [D, S], F32R, kind="ExternalInput")
    Wk = nc.dram_tensor("Wk", [EO, P, DO * P], F32R, kind="ExternalInput")
    Wq = nc.dram_tensor("Wq", [EO, P, DO * P], F32R, kind="ExternalInput")
    Wv = nc.dram_tensor("Wv", [D, D], F32R, kind="ExternalInput")
    bkT = nc.dram_tensor("bkT", [P, EO], F32, kind="ExternalInput")
    bqT = nc.dram_tensor("bqT", [P, EO], F32, kind="ExternalInput")
    bv = nc.dram_tensor("bv", [P, D], BF16, kind="ExternalInput")
    y = nc.dram_tensor("y", [HALF, D], F32, kind="ExternalOutput")

    xTr = xT.ap().rearrange("(do p) s -> p do s", p=P)
    Wvr = Wv.ap().rearrange("(do p) e -> p do e", p=P)

    with tile.TileContext(nc) as tc:
        with (
            tc.tile_pool(name="xts", bufs=2) as xts_pool,       # 32KB
            tc.tile_pool(name="wke", bufs=2) as wke_pool,       # 8KB
            tc.tile_pool(name="wve", bufs=1) as wve_pool,       # 32KB
            tc.tile_pool(name="kt", bufs=1) as k_pool,          # 32KB
            tc.tile_pool(name="qt", bufs=1) as q_pool,          # 32KB
            tc.tile_pool(name="vt", bufs=1) as v_pool,          # 32KB
            tc.tile_pool(name="pt", bufs=1) as p_pool,          # 32KB
            tc.tile_pool(name="outp", bufs=2) as out_pool,      # 4KB
            tc.tile_pool(name="small", bufs=1) as small_pool,
            tc.tile_pool(name="ps", bufs=6, space="PSUM") as ps_pool,
            tc.tile_pool(name="avz", bufs=1, space="PSUM") as avz_pool,
        ):
            bk_sb = small_pool.tile([P, EO], F32, tag="bk")
            bq_sb = small_pool.tile([P, EO], F32, tag="bq")
            bv_sb = small_pool.tile([P, D], BF16, tag="bv")
            ones_sb = small_pool.tile([P, 1], BF16, tag="ones")
            rz_sb = small_pool.tile([P, QT], F32, tag="rz")
            nc.vector.memset(ones_sb[:], 1.0)

            def emit_deferred_small_loads():
                nc.sync.dma_start(bk_sb[:], bkT[:, :])
                nc.sync.dma_start(bq_sb[:], bqT[:, :])
                nc.sync.dma_start(bv_sb[:], bv[:, :])

            # keep the PE busy (HAM warm) while the first x/W DMAs land
            warm_ps = avz_pool.tile([1, 8], F32, tag="warm")
            for _ in range(120):
                nc.tensor.matmul(
                    warm_ps[:, 0:1], ones_sb[:], ones_sb[:],
                    start=True, stop=True,
                )

            q_sb = q_pool.tile([P, EO, HALF], F32R, tag="qt")
            v_sb = v_pool.tile([P, KO, D], BF16, tag="vt")
            p_sb = p_pool.tile([P, KO, D], BF16, tag="pt")

            # ---- projections + scores, one k-half at a time --------------
            # query half is always s in [0, HALF) after the host swap.
            for kh in range(2):
                k_sb = k_pool.tile([P, EO, HALF], F32R, tag="kt")
                xq = []
                for sq in range(2):
                    t = xts_pool.tile([P, DO, 512], F32R, tag="xts")
                    for do in range(DO):   # split DMA across queues
                        nc.sync.dma_start(
                            t[:, do, :],
                            xTr[:, do, kh * HALF + sq * 512 : kh * HALF + sq * 512 + 512],
                        )
                    xq.append(t)
                if kh == 0:
                    emit_deferred_small_loads()
                # K^T (and Q^T in the query half): one contiguous wke load
                # per e-tile feeding both 512-wide s-quarters. The very first
                # e-tile runs its quarter-0 group before quarter 1 arrives.
                projs = [(Wk, bk_sb, k_sb)]
                if kh == 0:
                    projs.append((Wq, bq_sb, q_sb))
                for pi, (Wt, b_sb, dst_sb) in enumerate(projs):
                    for eo in range(EO):
                        wke = wke_pool.tile([P, DO, P], F32R, tag="wke")
                        nc.sync.dma_start(
                            wke[:], Wt[eo].unsqueeze(0).rearrange(
                                "o p (do e) -> (o p) do e", do=DO
                            ),
                        )
                        split_first = kh == 0 and pi == 0 and eo == 0
                        for sq in range(2):
                            ps0 = ps_pool.tile([P, 512], F32, tag="ps")
                            for do in range(DO):
                                nc.tensor.matmul(
                                    ps0[:], wke[:, do, :], xq[sq][:, do, :],
                                    start=(do == 0), stop=(do == DO - 1),
                                )
                            nc.vector.tensor_scalar_add(
                                dst_sb[:, eo, sq * 512 : sq * 512 + 512],
                                ps0[:],
                                b_sb[:, eo : eo + 1],
                            )
                        del split_first
                # V[s-half, :]: one ldweights per (ktl, do) serving both
                # 512-wide e-blocks; Wv d-slices resident for the half.
                wve = wve_pool.tile([P, DO, D], F32R, tag="wve")
                for do in range(DO):
                    nc.sync.dma_start(wve[:, do, :], Wvr[:, do, :])
                for ktl in range(8):
                    ko = kh * 8 + ktl
                    ps0 = ps_pool.tile([P, 512], F32, tag="ps")
                    ps1 = ps_pool.tile([P, 512], F32, tag="ps")
                    for do in range(DO):
                        xkt = xq[ktl // 4][:, do, (ktl % 4) * P : (ktl % 4) * P + P]
                        nc.tensor.matmul(
                            ps0[:], xkt, wve[:, do, 0:512],
                            start=(do == 0), stop=(do == DO - 1),
                        )
                        nc.tensor.matmul(
                            ps1[:], xkt, wve[:, do, 512:1024],
                            start=(do == 0), stop=(do == DO - 1),
                        )
                    nc.vector.tensor_tensor(
                        v_sb[:, ko, 0:512], ps0[:], bv_sb[:, 0:512],
                        mybir.AluOpType.add,
                    )
                    nc.vector.tensor_tensor(
                        v_sb[:, ko, 512:1024], ps1[:], bv_sb[:, 512:1024],
                        mybir.AluOpType.add,
                    )
                # scores^T for this k-half: exp(K^T.T @ Q^T / sqrt(D));
                # one ldweights per (ktl, eo) serving both q-blocks.
                for ktl in range(8):
                    ko = kh * 8 + ktl
                    ps0 = ps_pool.tile([P, 512], F32, tag="ps")
                    ps1 = ps_pool.tile([P, 512], F32, tag="ps")
                    for eo in range(EO):
                        kt_ap = k_sb[:, eo, ktl * P : (ktl + 1) * P]
                        nc.tensor.matmul(
                            ps0[:], kt_ap, q_sb[:, eo, 0:512],
                            start=(eo == 0), stop=(eo == EO - 1),
                        )
                        nc.tensor.matmul(
                            ps1[:], kt_ap, q_sb[:, eo, 512:1024],
                            start=(eo == 0), stop=(eo == EO - 1),
                        )
                    nc.scalar.activation(
                        p_sb[:, ko, 0:512], ps0[:],
                        mybir.ActivationFunctionType.Exp, scale=float(SCALE),
                    )
                    nc.scalar.activation(
                        p_sb[:, ko, 512:1024], ps1[:],
                        mybir.ActivationFunctionType.Exp, scale=float(SCALE),
                    )

            # ---- attention output: (P^T.T @ V) * (1/Z) -------------------
            zt = avz_pool.tile([P, QT], F32, tag="avz")
            for qt in range(QT):
                av0 = ps_pool.tile([P, 512], F32, tag="ps")
                av1 = ps_pool.tile([P, 512], F32, tag="ps")
                for ko in range(KO):
                    lhs = p_sb[:, ko, qt * P : (qt + 1) * P]
                    nc.tensor.matmul(
                        av0[:], lhs, v_sb[:, ko, 0:512],
                        start=(ko == 0), stop=(ko == KO - 1),
                    )
                    nc.tensor.matmul(
                        av1[:], lhs, v_sb[:, ko, 512:1024],
                        start=(ko == 0), stop=(ko == KO - 1),
                    )
                    nc.tensor.matmul(
                        zt[:, qt : qt + 1], lhs, ones_sb[:],
                        start=(ko == 0), stop=(ko == KO - 1),
                    )
                nc.vector.reciprocal(rz_sb[:, qt : qt + 1], zt[:, qt : qt + 1])
                o0 = out_pool.tile([P, 512], F32, tag="outp")
                o1 = out_pool.tile([P, 512], F32, tag="outp")
                nc.vector.tensor_scalar_mul(o0[:], av0[:], rz_sb[:, qt : qt + 1])
                nc.vector.tensor_scalar_mul(o1[:], av1[:], rz_sb[:, qt : qt + 1])
                nc.sync.dma_start(y[qt * P : (qt + 1) * P, 0:512], o0[:])
                nc.sync.dma_start(y[qt * P : (qt + 1) * P, 512:1024], o1[:])

    nc.finalize()
    return nc


_NC_CACHE = None


def make_in_maps(x, Wk, bk, Wq, bq, Wv, bv):
    import ml_dtypes

    x = np.asarray(x, dtype=np.float32)
    def _wre(W):
        # [D, D] -> [EO, P(part), DO*P] so each e-tile slice is one
        # fully contiguous per-partition DMA
        W = np.asarray(W, np.float32).reshape(DO, P, EO, P)
        return np.ascontiguousarray(
            W.transpose(2, 1, 0, 3).reshape(EO, P, DO * P)
        )

    Wk = _wre(Wk)
    Wq = _wre(Wq)
    Wv = np.ascontiguousarray(np.asarray(Wv, np.float32))
    bkT = np.ascontiguousarray(np.asarray(bk, np.float32).reshape(EO, P).T)
    bqT = np.ascontiguousarray(np.asarray(bq, np.float32).reshape(EO, P).T)
    bv2 = np.ascontiguousarray(
        np.broadcast_to(
            np.asarray(bv, np.float32).reshape(1, D), (P, D)
        ).astype(ml_dtypes.bfloat16)
    )

    in_maps = []
    for c in range(8):
        b, h = c // 2, c % 2
        xTb = np.ascontiguousarray(x[b].T)          # [D, S]
        if h == 1:
            # swap the s-halves so this core's query half is always first
            xTb = np.ascontiguousarray(
                np.concatenate([xTb[:, HALF:], xTb[:, :HALF]], axis=1)
            )
        in_maps.append(
            {
                "xT": xTb,
                "Wk": Wk, "Wq": Wq, "Wv": Wv,
                "bkT": bkT, "bqT": bqT, "bv": bv2,
            }
        )
    return in_maps


def gather_out(results):
    out = np.empty((B, S, D), dtype=np.float32)
    for c in range(8):
        b, h = c // 2, c % 2
        out[b, h * HALF : (h + 1) * HALF, :] = results[c]["y"]
    return out


def kernel(x, Wk, bk, Wq, bq, Wv, bv):
    global _NC_CACHE
    if _NC_CACHE is None:
        _NC_CACHE = build_nc()
    in_maps = make_in_maps(x, Wk, bk, Wq, bq, Wv, bv)
    res = run_bass_kernel_spmd(_NC_CACHE, in_maps, list(range(8)))
    return gather_out(res.results)
